# revision 43
# baseline (speedup 1.0000x reference)
"""Llama decoder layer on 8 Trainium2 NeuronCores.

v3 strategy (2x4 sharded attention + Megatron-TP MLP, all-bf16 matmuls):
  - Core c = (head-group i = c%2, token-group j = c//2): owns 16 heads x 512
    tokens for attention. All matmuls run N=512 moving rows (dense PE streams
    keep the HAM clock warm). K/V AllGather over same-i groups (4 cores);
    attention output AllGather over {2j, 2j+1} pairs; h2 AllGather over same-i;
    down-proj ReduceScatter over all 8 into 256-token output blocks.
  - Scores per pair: head A -> psum[:,0:512], head B -> psum[:,512:1024] (one
    2-bank tile), one exp per (pair,kc). AV uses a 65-column V lhsT (64 dims +
    ones) so softmax denominators fall out of the same matmul. Normalize =
    PE/gpsimd broadcast + wide fast-reciprocal + multiply.
  - The final residual (x2) is folded into the ReduceScatter input via a
    per-core {0,1} mask vector (only the i=0 core of each token group adds it),
    sidestepping SPMD core-dependent slicing.
"""
import os
import sys

sys.path.insert(0, "/opt/trn_rl_repo")

import numpy as np

import concourse.bacc as bacc
import concourse.mybir as mybir
import concourse.tile as tile
from concourse.bass_utils import run_bass_kernel_spmd

F32 = mybir.dt.float32
BF16 = mybir.dt.bfloat16
AF = mybir.ActivationFunctionType
MUL = mybir.AluOpType.mult
ADD = mybir.AluOpType.add

B, S, D, H = 1, 2048, 2048, 32
HD = D // H            # 64
FF = 8192
EPS = 1e-5
THETA = 10000.0
P = 128
NCORES = 8
SL = S // NCORES       # 256-token output block per core
SLT = 512              # local tokens (token-group)
HL = 16                # local heads (head-group)
NPL = HL // 2          # 8 local pairs
DLh = HL * HD          # 1024 local head dims
KH = DLh // P          # 8 chunks of local head dims
KD = D // P            # 16 chunks of D
FL = FF // NCORES      # 1024 local ff dims
KF = FL // P           # 8 chunks of local ff

G_KV = [[0, 2, 4, 6], [1, 3, 5, 7]]   # same head-group, ranks = token-group
G_AT = [[0, 1], [2, 3], [4, 5], [6, 7]]  # same token-group, ranks = head-group
G_ALL = [list(range(NCORES))]
G_H2X = [[0, 2], [1, 3], [4, 6], [5, 7]]  # token-group pairs within a half
G_RSH = [[0, 1, 2, 3], [4, 5, 6, 7]]      # token-half groups (ff quarters)
RS_CHUNKS = [6, 6, 3, 1]   # md counts per ReduceScatter chunk (small tail)


def _build():
    nc = bacc.Bacc(None, target_bir_lowering=False, num_devices=NCORES)

    xt = nc.dram_tensor("xt", [P, KD, SLT], BF16, kind="ExternalInput")
    xl = nc.dram_tensor("xl", [P, KD, SLT], F32, kind="ExternalInput")
    wq = nc.dram_tensor("wq", [NPL, P, KD, P], BF16, kind="ExternalInput")
    wk = nc.dram_tensor("wk", [KH, P, KD, P], BF16, kind="ExternalInput")
    wv = nc.dram_tensor("wv", [P, KD, DLh], BF16, kind="ExternalInput")
    wo = nc.dram_tensor("wo", [KD, P, KD, P], BF16, kind="ExternalInput")
    wg = nc.dram_tensor("wg", [P, KD, KF, P], BF16, kind="ExternalInput")
    wu = nc.dram_tensor("wu", [P, KD, KF, P], BF16, kind="ExternalInput")
    wd = nc.dram_tensor("wd", [P, KF, KD, P], BF16, kind="ExternalInput")
    cosl = nc.dram_tensor("cosl", [P, SLT], BF16, kind="ExternalInput")
    sinl = nc.dram_tensor("sinl", [P, SLT], BF16, kind="ExternalInput")
    perm = nc.dram_tensor("perm", [P, P], BF16, kind="ExternalInput")
    resm = nc.dram_tensor("resm", [P, 4], F32, kind="ExternalInput")
    out = nc.dram_tensor("out", [D, SL], F32, kind="ExternalOutput")

    with tile.TileContext(nc) as tc:
        with (
            tc.tile_pool(name="persist", bufs=1) as persist,
            tc.tile_pool(name="dram", bufs=1, space="DRAM") as dram,
        ):
            ones_bf = persist.tile([P, 1], BF16)
            nc.vector.memset(ones_bf, 1.0)
            ones1 = persist.tile([1, P], BF16)   # lhsT for PE row-broadcast
            nc.vector.memset(ones1, 1.0)
            # junk operands: filler matmuls keep the PE HAM clock warm through
            # ACT-gated stretches (result discarded)
            jw = persist.tile([P, P], BF16)
            nc.vector.memset(jw, 0.0)
            jr = persist.tile([P, 512], BF16)
            nc.vector.memset(jr, 0.0)
            perm_t = persist.tile([P, P], BF16)
            nc.sync.dma_start(perm_t, perm[:, :])
            eps_t = persist.tile([P, 1], F32)
            nc.vector.memset(eps_t, EPS)
            cosl_t = persist.tile([P, SLT], BF16)
            sinl_t = persist.tile([P, SLT], BF16)
            nc.sync.dma_start(cosl_t, cosl[:, :])
            nc.sync.dma_start(sinl_t, sinl[:, :])
            resm_t = persist.tile([P, 4], F32)
            nc.sync.dma_start(resm_t, resm[:, :])
            x2 = persist.tile([P, KD, SLT], F32)

            k_in = dram.tile([KH, P, SLT], BF16)
            v_inA = dram.tile([SLT, DLh // 2], BF16)
            v_inB = dram.tile([SLT, DLh // 2], BF16)
            k_outA = dram.tile([4, KH // 2, P, SLT], BF16)
            k_outB = dram.tile([4, KH // 2, P, SLT], BF16)
            v_outA = dram.tile([4, SLT, DLh // 2], BF16)
            v_outB = dram.tile([4, SLT, DLh // 2], BF16)
            at_in = dram.tile([KH, P, SLT], BF16)
            at_outA = dram.tile([2, KH // 2, P, SLT], BF16)
            at_outB = dram.tile([2, KH // 2, P, SLT], BF16)
            h2l = dram.tile([KD, P, SLT], BF16)
            h2g = dram.tile([NCORES, KD, P, SLT], BF16, addr_space="Shared")
            y_in = [dram.tile([NCORES, n, P, SL], BF16, name=f"y_in{r}")
                    for r, n in enumerate(RS_CHUNKS)]
            y_out = [dram.tile([n, P, SL], BF16, name=f"y_out{r}")
                     for r, n in enumerate(RS_CHUNKS)]

            with tc.tile_pool(name="pxl", bufs=1) as pxl:
                xl_t = pxl.tile([P, KD, SLT], F32)
                qTr = pxl.tile([HD, HL, SLT], BF16)

                # ================= phase 1: in-norm (local tokens) =============
                with tc.tile_pool(name="p1", bufs=1) as p1:
                    xh = p1.tile([P, KD, SLT], BF16)   # becomes h in place
                    for i in range(4):
                        nc.sync.dma_start(xh[:, 4 * i:4 * (i + 1)],
                                          xt[:, 4 * i:4 * (i + 1)])
                    h = xh
                    with (
                        tc.tile_pool(name="p1b", bufs=1) as p1b,
                        tc.tile_pool(name="ps1", bufs=1, space="PSUM") as ps1,
                    ):
                        ps_ms = ps1.tile([1, SLT], F32)
                        ps_j1 = ps1.tile([P, 512], F32)
                        for kc in range(KD):
                            sq = p1b.tile([P, SLT], BF16, tag="sq", bufs=2)
                            nc.vector.tensor_tensor(sq, xh[:, kc], xh[:, kc], MUL)
                            nc.tensor.matmul(ps_ms, ones_bf, sq,
                                             start=(kc == 0), stop=(kc == KD - 1))
                            nc.tensor.matmul(ps_j1, jw, jr, start=True, stop=True)
                            nc.tensor.matmul(ps_j1, jw, jr, start=True, stop=True)
                        std_bf = p1b.tile([1, SLT], BF16)
                        nc.scalar.activation(std_bf, ps_ms, AF.Sqrt,
                                             bias=eps_t[0:1], scale=1.0 / D)
                        ps_rbc = ps1.tile([P, SLT], F32)
                        nc.tensor.matmul(ps_rbc, ones1, std_bf, start=True, stop=True)
                        rbc = p1b.tile([P, SLT], F32)
                        nc.vector.reciprocal_approx_fast(rbc, ps_rbc)
                        for kc in range(KD):
                            nc.vector.tensor_tensor(xh[:, kc], xh[:, kc], rbc, MUL)

                    # ===== phase 2: K projection (16 local heads) + rope =====
                    with (
                        tc.tile_pool(name="p2", bufs=1) as p2,
                        tc.tile_pool(name="ps2", bufs=1, space="PSUM") as ps2,
                    ):
                        for m in range(KH):
                            wk_t = p2.tile([P, KD, P], BF16, tag="wk", bufs=2)
                            nc.sync.dma_start(wk_t, wk[m])
                            ps_k = ps2.tile([P, SLT], F32, tag="ps_k", bufs=2)
                            for kc in range(KD):
                                nc.tensor.matmul(ps_k, wk_t[:, kc], h[:, kc],
                                                 start=(kc == 0), stop=(kc == KD - 1))
                            kf = p2.tile([P, SLT], BF16, tag="kf", bufs=2)
                            nc.vector.tensor_copy(kf, ps_k)
                            tc_c = p2.tile([P, SLT], BF16, tag="tc_c", bufs=2)
                            nc.vector.tensor_tensor(tc_c, ps_k, cosl_t, MUL)
                            ps_rot = ps2.tile([P, SLT], F32, tag="ps_rot", bufs=2)
                            nc.tensor.matmul(ps_rot, perm_t, kf, start=True, stop=True)
                            ps_j2 = ps2.tile([P, 512], F32, tag="ps_j2", bufs=1)
                            nc.tensor.matmul(ps_j2, jw, jr, start=True, stop=True)
                            nc.tensor.matmul(ps_j2, jw, jr, start=True, stop=True)
                            ts_s = p2.tile([P, SLT], BF16, tag="ts_s", bufs=2)
                            nc.vector.tensor_tensor(ts_s, ps_rot, sinl_t, MUL)
                            kr = p2.tile([P, SLT], BF16, tag="kr", bufs=2)
                            nc.vector.tensor_tensor(kr, tc_c, ts_s, ADD)
                            nc.sync.dma_start(k_in[m], kr)
                            if m == KH // 2 - 1:
                                # first half of K ready: gather it early
                                nc.gpsimd.collective_compute(
                                    "AllGather", mybir.AluOpType.bypass,
                                    replica_groups=G_KV,
                                    ins=[k_in[0:KH // 2].opt()],
                                    outs=[k_outA.opt()],
                                )
                    nc.gpsimd.collective_compute(
                        "AllGather", mybir.AluOpType.bypass,
                        replica_groups=G_KV,
                        ins=[k_in[KH // 2:KH].opt()], outs=[k_outB.opt()],
                    )

                    # ===== phase 4: Q projection + rope (h doubles as hl) =====
                    with (
                        tc.tile_pool(name="p4", bufs=1) as p4,
                        tc.tile_pool(name="ps4", bufs=2, space="PSUM") as ps4,
                    ):
                        for pp in range(NPL):
                            wq_t = p4.tile([P, KD, P], BF16, tag="wq", bufs=2)
                            nc.sync.dma_start(wq_t, wq[pp])
                            ps_q = ps4.tile([P, SLT], F32, tag="ps_q")
                            for kc in range(KD):
                                nc.tensor.matmul(ps_q, wq_t[:, kc], h[:, kc],
                                                 start=(kc == 0), stop=(kc == KD - 1))
                            qf = p4.tile([P, SLT], BF16, tag="qf", bufs=2)
                            nc.vector.tensor_copy(qf, ps_q)
                            tc_q = p4.tile([P, SLT], BF16, tag="tc_q", bufs=2)
                            nc.vector.tensor_tensor(tc_q, ps_q, cosl_t, MUL)
                            ps_rq = ps4.tile([P, SLT], F32, tag="ps_rq")
                            nc.tensor.matmul(ps_rq, perm_t, qf, start=True, stop=True)
                            ps_j4 = ps4.tile([P, 512], F32, tag="ps_j4", bufs=1)
                            nc.tensor.matmul(ps_j4, jw, jr, start=True, stop=True)
                            nc.tensor.matmul(ps_j4, jw, jr, start=True, stop=True)
                            ts_q = p4.tile([P, SLT], BF16, tag="ts_q", bufs=2)
                            nc.vector.tensor_tensor(ts_q, ps_rq, sinl_t, MUL)
                            nc.vector.tensor_tensor(qTr[:, 2 * pp], tc_q[0:HD], ts_q[0:HD], ADD)
                            nc.vector.tensor_tensor(qTr[:, 2 * pp + 1], tc_q[HD:P], ts_q[HD:P], ADD)

                    # ===== phase 3: V projection (token-major out) =====
                    with (
                        tc.tile_pool(name="p3", bufs=1) as p3,
                        tc.tile_pool(name="ps3", bufs=2, space="PSUM") as ps3,
                    ):
                        wv_t = p3.tile([P, KD, DLh], BF16)
                        nc.sync.dma_start(wv_t, wv[:, :])
                        for dh in range(2):
                            v_dst = v_inA if dh == 0 else v_inB
                            for m in range(4):
                                ps_v = ps3.tile([P, 512], F32, tag="ps_v")
                                for kc in range(KD):
                                    nc.tensor.matmul(
                                        ps_v, h[:, kc, P * m:P * (m + 1)],
                                        wv_t[:, kc, 512 * dh:512 * (dh + 1)],
                                        start=(kc == 0), stop=(kc == KD - 1))
                                v_sb = p3.tile([P, 512], BF16, tag="v_sb", bufs=3)
                                nc.vector.tensor_copy(v_sb, ps_v)
                                ps_j3 = ps3.tile([P, 512], F32, tag="ps_j3", bufs=1)
                                nc.tensor.matmul(ps_j3, jw, jr, start=True, stop=True)
                                nc.sync.dma_start(v_dst[P * m:P * (m + 1), :], v_sb)
                            nc.gpsimd.collective_compute(
                                "AllGather", mybir.AluOpType.bypass,
                                replica_groups=G_KV,
                                ins=[(v_inA if dh == 0 else v_inB).opt()],
                                outs=[(v_outA if dh == 0 else v_outB).opt()],
                            )

                nc.sync.dma_start(xl_t, xl[:, :])  # residual, needed in phase 6

                # ============ phase 5: attention ============
                with (
                    tc.tile_pool(name="p5", bufs=1) as p5,
                    tc.tile_pool(name="ps5", bufs=1, space="PSUM") as ps5,
                ):
                    ps_j = ps5.tile([P, 512], F32, name="ps_j")
                    for pp in range(NPL):
                        kox = k_outA if pp < NPL // 2 else k_outB
                        ppx = pp % (NPL // 2)
                        kpA = p5.tile([HD, S], BF16, tag="kpA", bufs=2)
                        kpB = p5.tile([HD, S], BF16, tag="kpB", bufs=2)
                        for j4 in range(4):
                            nc.sync.dma_start(kpA[:, SLT * j4:SLT * (j4 + 1)],
                                              kox[j4, ppx, 0:HD, :])
                            nc.sync.dma_start(kpB[:, SLT * j4:SLT * (j4 + 1)],
                                              kox[j4, ppx, HD:P, :])
                        E = p5.tile([P, KD, 2 * SLT], BF16, tag="E", bufs=2)
                        for kc in range(KD):
                            ps_s = ps5.tile([P, 2 * SLT], F32, tag="ps_s", bufs=2)
                            nc.tensor.matmul(ps_s[:, 0:SLT], kpA[:, P * kc:P * (kc + 1)],
                                             qTr[:, 2 * pp], start=True, stop=True)
                            nc.tensor.matmul(ps_s[:, SLT:2 * SLT],
                                             kpB[:, P * kc:P * (kc + 1)],
                                             qTr[:, 2 * pp + 1], start=True, stop=True)
                            nc.tensor.matmul(ps_j, jw, jr, start=True, stop=True)
                            nc.scalar.activation(E[:, kc], ps_s, AF.Exp)
                        for hh in range(2):
                            hloc = 2 * pp + hh
                            vox = v_outA if hloc < HL // 2 else v_outB
                            hvx = hloc % (HL // 2)
                            vh = p5.tile([P, KD, HD + 1], BF16, tag="vh", bufs=3)
                            nc.vector.memset(vh[:, :, HD:HD + 1], 1.0)
                            for j4 in range(4):
                                nc.sync.dma_start(
                                    vh[:, 4 * j4:4 * (j4 + 1), 0:HD],
                                    vox[j4, :, HD * hvx:HD * (hvx + 1)]
                                    .rearrange("(kc p) f -> p kc f", p=P),
                                )
                            ps_av = ps5.tile([HD + 1, SLT], F32, tag="ps_av", bufs=2)
                            for kc in range(KD):
                                nc.tensor.matmul(ps_av, vh[:, kc],
                                                 E[:, kc, SLT * hh:SLT * (hh + 1)],
                                                 start=(kc == 0), stop=(kc == KD - 1))
                            dtmp = p5.tile([1, SLT], F32, tag="dtmp", bufs=2)
                            nc.vector.tensor_copy(dtmp, ps_av[HD:HD + 1])
                            dbc = p5.tile([HD, SLT], F32, tag="dbc", bufs=2)
                            nc.gpsimd.partition_broadcast(dbc, dtmp)
                            rcp = p5.tile([HD, SLT], F32, tag="rcp", bufs=2)
                            nc.vector.reciprocal_approx_fast(rcp, dbc)
                            atn = p5.tile([HD, SLT], BF16, tag="atn", bufs=3)
                            nc.vector.tensor_tensor(atn, ps_av[0:HD], rcp, MUL)
                            nc.sync.dma_start(
                                at_in[hloc // 2, HD * (hloc % 2):HD * (hloc % 2) + HD, :],
                                atn)
                        if pp == NPL // 2 - 1:
                            # first half of heads done: gather early
                            nc.gpsimd.collective_compute(
                                "AllGather", mybir.AluOpType.bypass,
                                replica_groups=G_AT,
                                ins=[at_in[0:KH // 2].opt()], outs=[at_outA.opt()],
                            )

                nc.gpsimd.collective_compute(
                    "AllGather", mybir.AluOpType.bypass,
                    replica_groups=G_AT,
                    ins=[at_in[KH // 2:KH].opt()], outs=[at_outB.opt()],
                )

                # ===== phase 6: out-proj + residual + post-norm =====
                with (
                    tc.tile_pool(name="p6", bufs=1) as p6,
                    tc.tile_pool(name="ps6", bufs=2, space="PSUM") as ps6,
                ):
                    at_f = p6.tile([P, KD, SLT], BF16)
                    for ii in range(2):
                        nc.sync.dma_start(at_f[:, KH * ii:KH * ii + KH // 2],
                                          at_outA[ii].rearrange("kc p t -> p kc t"))
                        nc.sync.dma_start(at_f[:, KH * ii + KH // 2:KH * (ii + 1)],
                                          at_outB[ii].rearrange("kc p t -> p kc t"))
                    for m in range(KD):
                        wo_t = p6.tile([P, KD, P], BF16, tag="wo", bufs=2)
                        nc.sync.dma_start(wo_t, wo[m])
                        ps_o = ps6.tile([P, SLT], F32, tag="ps_o")
                        for kc in range(KD):
                            nc.tensor.matmul(ps_o, wo_t[:, kc], at_f[:, kc],
                                             start=(kc == 0), stop=(kc == KD - 1))
                        nc.vector.tensor_tensor(x2[:, m], ps_o, xl_t[:, m], ADD)

                    ps_ms2 = ps6.tile([1, SLT], F32, bufs=1)
                    for m in range(KD):
                        sq2 = p6.tile([P, SLT], BF16, tag="sq2", bufs=2)
                        nc.scalar.square(sq2, x2[:, m])
                        nc.tensor.matmul(ps_ms2, ones_bf, sq2,
                                         start=(m == 0), stop=(m == KD - 1))
                    std2_bf = p6.tile([1, SLT], BF16)
                    nc.scalar.activation(std2_bf, ps_ms2, AF.Sqrt,
                                         bias=eps_t[0:1], scale=1.0 / D)
                    ps_rbc2 = ps6.tile([P, SLT], F32, bufs=1)
                    nc.tensor.matmul(ps_rbc2, ones1, std2_bf, start=True, stop=True)
                    rbc2 = p6.tile([P, SLT], F32)
                    nc.vector.reciprocal_approx_fast(rbc2, ps_rbc2)
                    h2sb = p6.tile([P, KD, SLT], BF16, name="h2sb")
                    for m in range(KD):
                        nc.vector.tensor_tensor(h2sb[:, m], x2[:, m], rbc2, MUL)
                        nc.sync.dma_start(h2l[m], h2sb[:, m])

            # xl/qTr freed; gather h2 across all cores (fast Shared path);
            # token-group pairs carry duplicate h2, so only even blocks are read
            nc.gpsimd.collective_compute(
                "AllGather", mybir.AluOpType.bypass,
                replica_groups=G_ALL, ins=[h2l.opt()], outs=[h2g.opt()],
            )
            # ===== phase 7: MLP (Megatron TP over ff dims, bf16) =====
            with (
                tc.tile_pool(name="p7", bufs=1) as p7,
                tc.tile_pool(name="ps7", bufs=1, space="PSUM") as ps7,
            ):
                h2f = p7.tile([P, KD, S], BF16)
                for jj in range(4):
                    nc.sync.dma_start(h2f[:, :, SLT * jj:SLT * (jj + 1)],
                                      h2g[2 * jj].rearrange("kc p t -> p kc t"))
                act = p7.tile([P, KF, S], BF16)
                ps_b = [ps7.tile([P, 512], F32, name=f"ps_b{i}", bufs=1)
                        for i in range(8)]
                for mf in range(KF):
                    wg_t = p7.tile([P, KD, P], BF16, tag="wgu", bufs=6)
                    nc.sync.dma_start(wg_t, wg[:, :, mf])
                    wu_t = p7.tile([P, KD, P], BF16, tag="wgu", bufs=6)
                    nc.sync.dma_start(wu_t, wu[:, :, mf])
                    for kc in range(KD):
                        for t in range(4):
                            nc.tensor.matmul(ps_b[t], wg_t[:, kc],
                                             h2f[:, kc, 512 * t:512 * (t + 1)],
                                             start=(kc == 0), stop=(kc == KD - 1))
                    for kc in range(KD):
                        for t in range(4):
                            nc.tensor.matmul(ps_b[4 + t], wu_t[:, kc],
                                             h2f[:, kc, 512 * t:512 * (t + 1)],
                                             start=(kc == 0), stop=(kc == KD - 1))
                    for t in range(4):
                        stmp = p7.tile([P, 512], F32, tag="stmp", bufs=3)
                        nc.scalar.activation(stmp, ps_b[4 + t], AF.Silu)
                        nc.vector.tensor_tensor(act[:, mf, 512 * t:512 * (t + 1)],
                                                ps_b[t], stmp, MUL)

                for md in range(KD):
                    pb = 4 * (md % 2)   # alternate psum banks so drains overlap
                    wd_t = p7.tile([P, KF, P], BF16, tag="wd", bufs=4)
                    nc.sync.dma_start(wd_t, wd[:, :, md])
                    for kf in range(KF):
                        for t in range(4):
                            nc.tensor.matmul(ps_b[pb + t], wd_t[:, kf],
                                             act[:, kf, 512 * t:512 * (t + 1)],
                                             start=(kf == 0), stop=(kf == KF - 1))
                    r = 0
                    base = 0
                    while md >= base + RS_CHUNKS[r]:
                        base += RS_CHUNKS[r]
                        r += 1
                    mdr = md - base
                    for t in range(4):
                        # fold the local residual x2 in via the per-core mask
                        yp = p7.tile([P, 2 * SL], BF16, tag="yp", bufs=3)
                        nc.vector.scalar_tensor_tensor(yp, x2[:, md],
                                                       resm_t[:, t:t + 1],
                                                       ps_b[pb + t], MUL, ADD)
                        nc.sync.dma_start(y_in[r][2 * t, mdr], yp[:, 0:SL])
                        nc.sync.dma_start(y_in[r][2 * t + 1, mdr], yp[:, SL:2 * SL])
                    if mdr == RS_CHUNKS[r] - 1:
                        nc.gpsimd.collective_compute(
                            "ReduceScatter", mybir.AluOpType.add,
                            replica_groups=G_ALL,
                            ins=[y_in[r].opt()], outs=[y_out[r].opt()],
                        )

            with tc.tile_pool(name="p8", bufs=1) as p8:
                md = 0
                for r, nmd in enumerate(RS_CHUNKS):
                    for mdr in range(nmd):
                        ys = p8.tile([P, SL], BF16, tag="ys", bufs=3)
                        nc.sync.dma_start(ys, y_out[r][mdr])
                        o_sb = p8.tile([P, SL], F32, tag="o_sb", bufs=3)
                        nc.vector.tensor_copy(o_sb, ys)
                        nc.sync.dma_start(out[P * md:P * (md + 1), :], o_sb)
                        md += 1
    nc.compile()
    return nc


_NC_CACHE = {}


def _get_nc():
    if "nc" not in _NC_CACHE:
        _NC_CACHE["nc"] = _build()
    return _NC_CACHE["nc"]


def _rope_tables():
    inv_freq = (1.0 / (THETA ** (np.arange(0, HD, 2, dtype=np.float32) / HD))).astype(np.float32)
    pos = np.arange(S, dtype=np.float32)
    freqs = pos[:, None] * inv_freq[None, :]                  # [S, HD/2]
    emb = np.concatenate([freqs, freqs], axis=-1)             # [S, HD]
    cos = np.cos(emb).astype(np.float32)
    sin = np.sin(emb).astype(np.float32)
    cosT = cos.T                                              # [HD, S]
    sinT = sin.T.copy()
    sinT[0:HD // 2] *= -1.0                                   # sign folded for rotate_half
    return np.tile(cosT, (2, 1)), np.tile(sinT, (2, 1))       # [128, S] each


def _perm_matrix():
    # lhsT for rotate-half shift: out[m] = q[(m+32) % 64 within each 64 block]
    pm = np.zeros((P, P), np.float32)
    for m in range(P):
        blk = (m // HD) * HD
        src = blk + (m - blk + HD // 2) % HD
        pm[src, m] = 1.0
    return pm


def _tile_lhsT(wT, n_m):
    # [Kdim, Mdim] -> [m, p, kc, f] blocks for SBUF lhsT tiles
    Kdim, Mdim = wT.shape
    kc = Kdim // P
    return np.ascontiguousarray(
        wT.reshape(kc, P, n_m, Mdim // n_m).transpose(2, 1, 0, 3))


def _prep_in_maps(inputs):
    import ml_dtypes
    bf16 = ml_dtypes.bfloat16

    x = np.asarray(inputs["x"], np.float32)
    w_in = np.asarray(inputs["w_in_norm"], np.float32)
    wq = np.asarray(inputs["wq"], np.float32)
    wk = np.asarray(inputs["wk"], np.float32)
    wv = np.asarray(inputs["wv"], np.float32)
    wo = np.asarray(inputs["wo"], np.float32)
    w_post = np.asarray(inputs["w_post_norm"], np.float32)
    wg = np.asarray(inputs["wg"], np.float32)
    wu = np.asarray(inputs["wu"], np.float32)
    wd = np.asarray(inputs["wd"], np.float32)

    xT = np.ascontiguousarray(x[0].T)                         # [D, S]
    xt_sb = np.ascontiguousarray(xT.reshape(KD, P, S).transpose(1, 0, 2))  # [p, kc, s]

    scale = 1.0 / np.sqrt(HD)
    wq_eff_T = np.ascontiguousarray((wq * w_in[None, :]).T * scale)  # [D, D]
    wk_eff_T = np.ascontiguousarray((wk * w_in[None, :]).T)
    wv_eff_T = np.ascontiguousarray((wv * w_in[None, :]).T)
    wo_sb = _tile_lhsT(np.ascontiguousarray(wo.T), KD).astype(bf16)

    wgT = np.ascontiguousarray((wg * w_post[None, :]).T)      # [D, FF]
    wuT = np.ascontiguousarray((wu * w_post[None, :]).T)
    wdT = np.ascontiguousarray(wd.T)                          # [FF, D]

    cosT, sinT = _rope_tables()
    cosT_bf = cosT.astype(bf16)
    sinT_bf = sinT.astype(bf16)
    pm = _perm_matrix().astype(bf16)

    nc = _get_nc()
    in_maps = []
    for c in range(NCORES):
        i, j = c % 2, c // 2
        tok = slice(SLT * j, SLT * (j + 1))
        hd_sl = slice(DLh * i, DLh * (i + 1))
        wq_sb = _tile_lhsT(wq_eff_T[:, hd_sl], NPL).astype(bf16)   # [8,128,16,128]
        wk_sb = _tile_lhsT(wk_eff_T[:, hd_sl], KH).astype(bf16)    # [8,128,16,128]
        wv_sb = np.ascontiguousarray(
            wv_eff_T[:, hd_sl].reshape(KD, P, DLh).transpose(1, 0, 2)).astype(bf16)
        ff_sl = slice(FL * c, FL * (c + 1))
        wg_sb = np.ascontiguousarray(
            wgT[:, ff_sl].reshape(KD, P, KF, P).transpose(1, 0, 2, 3)).astype(bf16)
        wu_sb = np.ascontiguousarray(
            wuT[:, ff_sl].reshape(KD, P, KF, P).transpose(1, 0, 2, 3)).astype(bf16)
        wd_sb = np.ascontiguousarray(
            wdT[ff_sl, :].reshape(KF, P, KD, P).transpose(1, 0, 2, 3)).astype(bf16)
        resmv = np.zeros((P, 4), np.float32)
        if i == 0:
            resmv[:, j] = 1.0
        in_maps.append({
            "xt": xt_sb[:, :, tok].astype(bf16),
            "xl": np.ascontiguousarray(xt_sb[:, :, tok]),
            "wq": wq_sb, "wk": wk_sb, "wv": wv_sb, "wo": wo_sb,
            "wg": wg_sb, "wu": wu_sb, "wd": wd_sb,
            "cosl": np.ascontiguousarray(cosT_bf[:, tok]),
            "sinl": np.ascontiguousarray(sinT_bf[:, tok]),
            "perm": pm,
            "resm": resmv,
        })
    return in_maps


def kernel(**inputs):
    nc = _get_nc()
    in_maps = _prep_in_maps(inputs)
    res = run_bass_kernel_spmd(
        nc, in_maps, core_ids=list(range(NCORES)),
        trace=bool(os.environ.get("KERNEL_TRACE")),
    )
    _NC_CACHE["last_result"] = res

    full = np.empty((B, S, D), np.float32)
    for c in range(NCORES):
        full[0, c * SL:(c + 1) * SL, :] = res.results[c]["out"].T
    return full
